# revision 14
# baseline (speedup 1.0000x reference)
"""CPAttention Trainium2 kernel: 8-way batch-data-parallel over 8 NeuronCores.

v3.2: single-head attention loop, reduction-free score/Z plumbing, and
hi/lo-split bf16 dots.
  - qkproj fp32 (score fidelity)
  - dots: d = (kh+kl)*qh + kh*ql in two bf16 passes (K=128 pack + K=64),
    dropping only the ~2^-18 kl*ql term; full fp32-class precision for the
    argsort-critical score at bf16 matmul speed
  - score: A[jt] += |dots*mask| accumulated over heads (abs on V/S, adds
    on GpSimd), reduced by 8 fp32 ones-matmuls at the end (interleaved
    into the last head)
  - Z: rides the AV matmul as a 65th ones-column of V; 1/Z broadcast via
    GpSimd partition_broadcast (no PE, no extra PSUM)
  - AV emitted with one-iteration lag ("carry") so the PE never stalls on
    the exp -> keeps the PE p-state high
Host applies the argsort + 16-step row swap (commutes with w_out).
"""
import numpy as np

import concourse.bacc as bacc
import concourse.tile as tile
from concourse import mybir
from concourse.bass_utils import run_bass_kernel_spmd

F32 = mybir.dt.float32
BF16 = mybir.dt.bfloat16
U32 = mybir.dt.uint32
AOP = mybir.AluOpType
AFT = mybir.ActivationFunctionType

B, N, DIM = 8, 1024, 512
HEADS, DH = 8, 64
INNER = 512
SCALE = DH ** -0.5

_cache = {}


def _build():
    nc = bacc.Bacc()
    xT = nc.declare_dram_parameter("xT", [DIM, N], F32, isOutput=False)
    xTbf = nc.declare_dram_parameter("xTbf", [DIM, N], BF16, isOutput=False)
    maskT = nc.declare_dram_parameter("maskT", [N, N], BF16, isOutput=False)
    wqk = nc.declare_dram_parameter("wqk", [DIM, 2 * INNER], F32, isOutput=False)
    wvbf = nc.declare_dram_parameter("wvbf", [DIM, INNER], BF16, isOutput=False)
    wobf = nc.declare_dram_parameter("wobf", [INNER, DIM], BF16, isOutput=False)
    bout = nc.declare_dram_parameter("bout", [1, DIM], F32, isOutput=False)
    y_out = nc.declare_dram_parameter("y", [N, DIM], F32, isOutput=True)
    sc_out = nc.declare_dram_parameter("score", [1, N], F32, isOutput=True)

    with tile.TileContext(nc) as tc:
        with tc.tile_pool(name="cst", bufs=1) as cst, \
             tc.tile_pool(name="stage", bufs=1) as stage, \
             tc.tile_pool(name="wrk", bufs=3) as wrk, \
             tc.tile_pool(name="wrkta", bufs=3) as wrkta, \
             tc.tile_pool(name="wrk4", bufs=4) as wrk4, \
             tc.tile_pool(name="stg", bufs=2) as stg, \
             tc.tile_pool(name="zp", bufs=2) as zp, \
             tc.tile_pool(name="eph", bufs=2) as eph, \
             tc.tile_pool(name="one", bufs=1) as one, \
             tc.tile_pool(name="ppA", bufs=1, space="PSUM") as ppA, \
             tc.tile_pool(name="ppB", bufs=1, space="PSUM") as ppB, \
             tc.tile_pool(name="pvA", bufs=1, space="PSUM") as pvA, \
             tc.tile_pool(name="pvB", bufs=1, space="PSUM") as pvB:

            # ---- loads ----
            xt_t = []
            wq_t = []
            for kt in range(4):
                xk = stage.tile([128, N], F32, tag=f"xt{kt}")
                nc.sync.dma_start(out=xk, in_=xT[kt * 128:(kt + 1) * 128, :])
                wk = stage.tile([128, 2 * INNER], F32, tag=f"wq{kt}")
                nc.sync.dma_start(out=wk[:, 0:512],
                                  in_=wqk[kt * 128:(kt + 1) * 128, 0:512])
                nc.sync.dma_start(out=wk[:, 512:1024],
                                  in_=wqk[kt * 128:(kt + 1) * 128, 512:1024])
                xt_t.append(xk)
                wq_t.append(wk)
            xtb = cst.tile([128, 4, N], BF16)
            nc.sync.dma_start(out=xtb, in_=xTbf[:, :].rearrange("(t p) i -> p t i", p=128))
            msk = cst.tile([128, 8, N], BF16)
            nc.sync.dma_start(out=msk, in_=maskT[:, :].rearrange("(t p) i -> p t i", p=128))
            wvb = cst.tile([128, 4, INNER], BF16)
            nc.sync.dma_start(out=wvb, in_=wvbf[:, :].rearrange("(t p) c -> p t c", p=128))
            wob = cst.tile([128, 4, DIM], BF16)
            nc.sync.dma_start(out=wob, in_=wobf[:, :].rearrange("(t p) e -> p t e", p=128))
            bb = cst.tile([128, DIM], F32)
            nc.sync.dma_start(out=bb, in_=bout[0:1, :].to_broadcast([128, DIM]))

            ones32 = cst.tile([128, 1], F32)
            nc.vector.memset(ones32, 1.0)
            onesbf = cst.tile([128, 1], BF16)
            nc.vector.memset(onesbf, 1.0)

            vv = cst.tile([128, HEADS, 8, DH + 1], BF16)
            nc.vector.memset(vv[:, :, :, DH:DH + 1], 1.0)

            # hi/lo dot-product surfaces, per head (bf16):
            #   khl:  rows 0:64 = k_hi, rows 64:128 = k_lo
            #   qhh:  rows 0:64 = q_hi, rows 64:128 = q_hi (duplicate)
            #   qlx:  rows 0:64 = q_lo
            khl = cst.tile([128, HEADS, N], BF16)
            qhh = cst.tile([128, HEADS, N], BF16)
            qlx = cst.tile([64, HEADS, N], BF16)
            onorm = cst.tile([128, 4, N], BF16)

            # ---- V projection (bf16) ----
            for jt in range(8):
                pool, tag = (ppA, "dA") if jt % 2 == 0 else (ppB, "dB")
                pv = pool.tile([128, INNER], F32, tag=tag)
                for kt in range(4):
                    nc.tensor.matmul(
                        pv,
                        xtb[:, kt, jt * 128:(jt + 1) * 128],
                        wvb[:, kt, :],
                        start=(kt == 0), stop=(kt == 3))
                nc.scalar.activation(
                    out=vv[:, :, jt, 0:DH],
                    in_=pv.rearrange("p (h d) -> p h d", h=HEADS),
                    func=AFT.Copy)

            # ---- nnz ----
            nzp = pvA.tile([1, N], F32, tag="vA")
            for jt in range(8):
                for ic in range(2):
                    sl = slice(ic * 512, (ic + 1) * 512)
                    nc.tensor.matmul(nzp[:, sl], onesbf, msk[:, jt, sl],
                                     start=(jt == 0), stop=(jt == 7),
                                     skip_group_check=True)
            scr = one.tile([1, N], F32, tag="scr")
            rnz = one.tile([1, N], F32, tag="rnz")
            nc.vector.reciprocal_approx_accurate(out=rnz, in_=nzp, scratch=scr)

            # ---- QK projection (fp32) + hi/lo extraction ----
            # k-blocks (ct 4..7) interleaved first so head surfaces finish early
            for ct in [4, 0, 5, 1, 6, 2, 7, 3]:
                pool, tag = (ppA, "dA") if ct % 2 == 0 else (ppB, "dB")
                pq = pool.tile([128, N], F32, tag=tag)
                for ic in range(2):
                    sl = slice(ic * 512, (ic + 1) * 512)
                    for kt in range(4):
                        nc.tensor.matmul(
                            pq[:, sl],
                            wq_t[kt][:, ct * 128:(ct + 1) * 128],
                            xt_t[kt][:, sl],
                            start=(kt == 0), stop=(kt == 3),
                            skip_group_check=True)
                if ct < 4:
                    hA, hB = 2 * ct, 2 * ct + 1
                    # head A (rows 0:64): q_hi direct + dup shift; q_lo direct
                    nc.scalar.activation(out=qhh[0:64, hA, :], in_=pq[0:64, :],
                                         func=AFT.Copy)
                    nc.sync.dma_start(out=qhh[64:128, hA, :],
                                      in_=qhh[0:64, hA, :])
                    nc.vector.tensor_tensor(out=qlx[0:64, hA, :],
                                            in0=pq[0:64, :],
                                            in1=qhh[0:64, hA, :],
                                            op=AOP.subtract)
                    # head B (rows 64:128): q_hi direct + dup shift;
                    # q_lo via staging + shift
                    nc.scalar.activation(out=qhh[64:128, hB, :],
                                         in_=pq[64:128, :], func=AFT.Copy)
                    nc.sync.dma_start(out=qhh[0:64, hB, :],
                                      in_=qhh[64:128, hB, :])
                    sq = stg.tile([128, N], BF16, tag="sq")
                    nc.vector.tensor_tensor(out=sq[64:128, :],
                                            in0=pq[64:128, :],
                                            in1=qhh[64:128, hB, :],
                                            op=AOP.subtract)
                    nc.sync.dma_start(out=qlx[0:64, hB, :], in_=sq[64:128, :])
                else:
                    hA, hB = 2 * (ct - 4), 2 * (ct - 4) + 1
                    # head A (rows 0:64): k_hi direct; k_lo via staging + shift
                    nc.scalar.activation(out=khl[0:64, hA, :], in_=pq[0:64, :],
                                         func=AFT.Copy)
                    sk = stg.tile([128, N], BF16, tag="sq")
                    nc.vector.tensor_tensor(out=sk[0:64, :], in0=pq[0:64, :],
                                            in1=khl[0:64, hA, :],
                                            op=AOP.subtract)
                    nc.sync.dma_start(out=khl[64:128, hA, :], in_=sk[0:64, :])
                    # head B (rows 64:128): k_hi via staging + shift; k_lo direct
                    sk2 = stg.tile([128, N], BF16, tag="sk2")
                    nc.scalar.activation(out=sk2[64:128, :], in_=pq[64:128, :],
                                         func=AFT.Copy)
                    nc.sync.dma_start(out=khl[0:64, hB, :], in_=sk2[64:128, :])
                    nc.vector.tensor_tensor(out=khl[64:128, hB, :],
                                            in0=pq[64:128, :],
                                            in1=sk2[64:128, :],
                                            op=AOP.subtract)

            # score accumulator tiles: reuse xt/wq stage slots (dead after qkproj)
            A_t = []
            for j in range(8):
                atag = f"xt{j}" if j < 4 else f"wq{j - 4}"
                ajt = stage.tile([128, N], F32, tag=atag, name=f"A{j}")
                A_t.append(ajt)

            # ---- attention, per head; AV emitted with one-iteration lag ----
            def emit_av(av_c, h_c, jt_c, es_c):
                for ic in range(2):
                    sl = slice(ic * 512, (ic + 1) * 512)
                    nc.tensor.matmul(av_c[:, sl], vv[:, h_c, jt_c, :],
                                     es_c[:, sl],
                                     start=(jt_c == 0), stop=(jt_c == 7),
                                     skip_group_check=True)

            def emit_norm(av_c, h_c):
                po_c = (h_c % 2) * 64
                zrow = zp.tile([1, N], F32, tag="zrow")
                nc.scalar.activation(out=zrow, in_=av_c[DH:DH + 1, :],
                                     func=AFT.Copy)
                zr = zp.tile([1, N], F32, tag="zr")
                nc.vector.reciprocal_approx_fast(out=zr, in_=zrow)
                zbh = zp.tile([128, N], F32, tag="zb")
                nc.gpsimd.partition_broadcast(zbh, zr)
                nc.vector.tensor_tensor(
                    out=onorm[po_c:po_c + 64, h_c // 2, :], in0=av_c[0:DH, :],
                    in1=zbh[po_c:po_c + 64, :], op=AOP.mult)

            scp_box = []

            def emit_score_reduce(jt_c):
                if not scp_box:
                    scp_box.append(pvA.tile([1, N], F32, tag="vA", name="scp"))
                scp_c = scp_box[0]
                for ic in range(2):
                    sl = slice(ic * 512, (ic + 1) * 512)
                    nc.tensor.matmul(scp_c[:, sl], ones32, A_t[jt_c][:, sl],
                                     start=(jt_c == 0), stop=(jt_c == 7),
                                     skip_group_check=True)

            avs = {}
            carry = None
            for h in range(HEADS):
                avpool, avtag = (pvA, "vA") if h % 2 == 0 else (pvB, "vB")
                av = avpool.tile([DH + 1, N], F32, tag=avtag, name=f"av{h}")
                avs[h] = av
                for jt in range(8):
                    dpool, dtag = (ppA, "dA") if jt % 2 == 0 else (ppB, "dB")
                    d = dpool.tile([128, N], F32, tag=dtag)
                    jb = slice(jt * 128, (jt + 1) * 128)
                    for ic in range(2):
                        sl = slice(ic * 512, (ic + 1) * 512)
                        nc.tensor.matmul(
                            d[:, sl], khl[:, h, jb], qhh[:, h, sl],
                            start=True, stop=False,
                            skip_group_check=True)
                        nc.tensor.matmul(
                            d[:, sl], khl[0:64, h, jb], qlx[0:64, h, sl],
                            start=False, stop=True, tile_position=(0, 0),
                            skip_group_check=True)
                    if carry is not None:
                        ch, cjt, ces = carry
                        emit_av(avs[ch], ch, cjt, ces)
                        if cjt == 7:
                            emit_norm(avs[ch], ch)
                    if h == 7 and jt >= 2:
                        emit_score_reduce(jt - 2)
                    t = wrk.tile([128, N], F32, tag="t")
                    nc.vector.tensor_tensor(out=t, in0=d, in1=msk[:, jt, :],
                                            op=AOP.mult)
                    es = wrk4.tile([128, N], BF16, tag="e")
                    nc.scalar.activation(out=es, in_=t, func=AFT.Exp, scale=SCALE)
                    carry = (h, jt, es)
                    if h == 0:
                        nc.scalar.activation(out=A_t[jt], in_=t, func=AFT.Abs)
                    else:
                        ta = wrkta.tile([128, N], F32, tag="ta")
                        if h in (1, 2, 3):
                            nc.vector.tensor_scalar(
                                out=ta.bitcast(U32), in0=t.bitcast(U32),
                                scalar1=0x7FFFFFFF, scalar2=None,
                                op0=AOP.bitwise_and)
                        else:
                            nc.scalar.activation(out=ta, in_=t, func=AFT.Abs)
                        nc.gpsimd.tensor_tensor(out=A_t[jt], in0=ta,
                                                in1=A_t[jt], op=AOP.add)
            ch, cjt, ces = carry
            emit_av(avs[ch], ch, cjt, ces)
            emit_norm(avs[ch], ch)
            for jt in (6, 7):
                emit_score_reduce(jt)

            sc_sb = one.tile([1, N], F32, tag="scr")
            nc.vector.scalar_tensor_tensor(
                out=sc_sb, in0=scp_box[0], scalar=SCALE, in1=rnz,
                op0=AOP.mult, op1=AOP.mult)
            nc.gpsimd.dma_start(out=sc_out[:, :], in_=sc_sb)

            # ---- output projection (bf16) ----
            for it in range(8):
                pool, tag = (ppB, "dB") if it % 2 == 0 else (ppA, "dA")
                yp = pool.tile([128, DIM], F32, tag=tag)
                for pr in range(4):
                    nc.tensor.matmul(
                        yp,
                        onorm[:, pr, it * 128:(it + 1) * 128],
                        wob[:, pr, :],
                        start=(pr == 0), stop=(pr == 3))
                yt = eph.tile([128, DIM], F32, tag="yt")
                nc.vector.tensor_tensor(out=yt, in0=yp, in1=bb, op=AOP.add)
                nc.sync.dma_start(out=y_out[it * 128:(it + 1) * 128, :], in_=yt)
    nc.finalize()
    return nc


def _get_nc():
    if "nc" not in _cache:
        _cache["nc"] = _build()
    return _cache["nc"]


def _run_device(inputs, trace=False):
    x = np.asarray(inputs["x"], np.float32)
    cp_mask = np.asarray(inputs["cp_mask"])
    w_qkv = np.asarray(inputs["w_qkv"], np.float32)
    w_out = np.asarray(inputs["w_out"], np.float32)
    b_out = np.asarray(inputs["b_out"], np.float32)

    bf = mybir.dt.np(BF16)
    maskT = np.ascontiguousarray(cp_mask.T).astype(bf)
    wqk = np.ascontiguousarray(w_qkv[:, :2 * INNER])
    wvbf = np.ascontiguousarray(w_qkv[:, 2 * INNER:]).astype(bf)
    wobf = np.ascontiguousarray(w_out).astype(bf)
    boutr = np.ascontiguousarray(b_out.reshape(1, DIM))

    in_maps = []
    for b in range(B):
        xTb = np.ascontiguousarray(x[b].T)
        in_maps.append({
            "xT": xTb,
            "xTbf": xTb.astype(bf),
            "maskT": maskT,
            "wqk": wqk,
            "wvbf": wvbf,
            "wobf": wobf,
            "bout": boutr,
        })

    nc = _get_nc()
    res = run_bass_kernel_spmd(nc, in_maps, core_ids=list(range(B)), trace=trace)
    y = np.stack([res.results[b]["y"] for b in range(B)])
    score = np.stack([res.results[b]["score"][0] for b in range(B)])
    return y, score, res


def _apply_swap(y, score, patches):
    idx = np.argsort(score, axis=-1, kind="stable")[::-1]
    out = y.copy()
    clone = y
    bi = np.arange(B)
    for i in range(1, patches + 1):
        ti = idx[:, i]
        out[bi, i] = clone[bi, ti]
        out[bi, ti] = clone[:, i]
    return out


def kernel(**inputs):
    patches = int(np.asarray(inputs["patches_in_core_nodes"]))
    y, score, _ = _run_device(inputs, trace=False)
    return _apply_swap(y, score, patches)
